# revision 40
# baseline (speedup 1.0000x reference)
"""Trainium2 Bass kernel for nn_ActorCriticGAT (2-layer GATv2 + global-mean-pool
actor-critic head), distributed over 8 NeuronCores.

Optimizations over the v2 baseline (1134us -> ~930us):
 - no ag1 collective: every core computes the full xl1 gather table locally
   from the replicated x input (162 tiles of x @ W1l, partition-major table
   layout so the 27 batched DRAM writes are contiguous per partition).
 - ag2 pipelined: xl2 is AllGathered in 3 pieces (7 windows each) issued as
   soon as each piece's windows finish layer-1; xl2_full uses a piece-major
   row layout and the layer-2 gather indices are remapped host-side.
 - gather tables in fp8e4 (halves gather DMA + collective bytes; ~3% value
   quantization washes out through softmax normalization + mean-pool).
 - Mm/Dm merged into one partition-major fp8 DRAM tensor (one contiguous
   load per window; one-hot entries exact in fp8).
 - gathers stay 4-way split across the 4 SWDGE queues (single merged
   gathers serialize their transfer and cost 2x on the GpSimd engine).
 - exp activation writes bf16 directly (no f32 CAST pass on the DVE).
 - final pool AllReduce kept minimal (65KB); its ~77us cost is core-skew
   bound (AllGather/AllToAll variants measured identical).

Sharding: nodes are padded to 8*21*123 = 20664 and split into 128-edge chunks
grouped by 123-node destination windows; each core owns 21 windows (2583 dst
nodes) and all edges pointing into them.  Per-edge work is edge-major
(edges on the SBUF partition axis):

  s    = xl1[src] + xr1[dst] + edge_attr @ W1e        (PSUM, 2 matmuls)
  m    = leaky_relu(s, 0.2)                           (ScalarE Prelu)
  lg   = att . m  (per head)                          (DVE mul+reduce)
  p    = exp(lg)  (softmax without max-subtraction: logits are O(1))
  num  = sum_e p * xl1[src],  den = sum_e p           (PE one-hot scatter matmul)
  h    = relu(num / den)
"""

import os
import sys

sys.path.insert(0, "/opt/trn_rl_repo")

import numpy as np
import ml_dtypes
from contextlib import ExitStack
from dataclasses import dataclass, field

# ---------------------------------------------------------------- problem dims
N, E, G = 20000, 320000, 64
IN, HID, HEADS, EDGE_D, ACT = 128, 256, 4, 5, 8
NEG_SLOPE = 0.2

NCORES = 8
NW = 128 - EDGE_D  # 123 dst nodes per window (+5 rows of edge_attr = K=128)
WPC = 21  # windows per core
NPC = NW * WPC  # 2583 nodes per core
NPAD = NCORES * NPC  # 20664 padded nodes
NT = (NPAD + 127) // 128  # 162 xl1-table tiles
NTAB = NT * 128  # 20736 rows in the xl1 gather table
PB = [0, 7, 14, 21]  # ag2 piece boundaries (windows)
PIECES = len(PB) - 1
TBATCH = 6  # xl1-table tiles per batched DRAM write (162 = 27 batches)
LG_BF16 = os.environ.get("KERNEL_LG_F32", "0") != "1"


@dataclass
class Cfg:
    n: int = N
    e: int = E
    g: int = G
    ncores: int = NCORES
    wpc: int = WPC
    cwins: list = field(default_factory=list)  # chunks per window slot [wpc]
    debug_taps: bool = False
    stop_after: str = "full"  # p1 | l1 | l2 | full

    @property
    def npc(self):
        return NW * self.wpc

    @property
    def npad(self):
        return self.ncores * self.npc

    @property
    def nch(self):
        return sum(self.cwins)

    @property
    def cwmax(self):
        return max(self.cwins)


def _bf(x):
    return np.asarray(x, dtype=ml_dtypes.bfloat16)


def _f8(x):
    return np.asarray(x, dtype=ml_dtypes.float8_e4m3fn)


def _f32(x):
    return np.ascontiguousarray(x, dtype=np.float32)


# ================================================================ host side
def preprocess(cfg: Cfg, x, edge_index, batch, edge_attr):
    """Build per-core input maps. Returns in_maps (cfg.cwins filled)."""
    src = np.asarray(edge_index[0], dtype=np.int64)
    dst = np.asarray(edge_index[1], dtype=np.int64)
    ea = _f32(edge_attr)
    batch = np.asarray(batch, dtype=np.int64)

    nwin_total = cfg.ncores * cfg.wpc
    win = dst // NW  # window id of each edge
    order = np.argsort(win, kind="stable")
    win_sorted = win[order]
    bounds = np.searchsorted(win_sorted, np.arange(nwin_total + 1))

    # chunks per window slot (equal across cores for SPMD)
    cw = np.zeros(cfg.wpc, dtype=np.int64)
    for w in range(nwin_total):
        cnt = bounds[w + 1] - bounds[w]
        slot = w % cfg.wpc
        cw[slot] = max(cw[slot], (cnt + 127) // 128)
    cw = np.maximum(cw, 1)
    cfg.cwins = [int(c) for c in cw]

    nch = cfg.nch
    cum = np.concatenate([[0], np.cumsum(cw)])  # chunk offset of each window slot

    # layer-2 table row of global node id s (piece-major layout)
    pbv = np.asarray(PB)

    def l2row(s):
        r = s // cfg.npc
        rem = s % cfg.npc
        w = rem // NW
        i = rem % NW
        p = np.searchsorted(pbv, w, side="right") - 1
        wp = pbv[p + 1] - pbv[p]  # windows in piece p
        return (
            cfg.ncores * NW * pbv[p]
            + r * (wp * NW)
            + (w - pbv[p]) * NW
            + i
        )

    # full padded x^T (replicated on every core)
    xf = np.zeros((NTAB, IN), dtype=np.float32)
    xf[: cfg.n] = np.asarray(x, dtype=np.float32)
    xTf = _bf(xf.T)

    # layer-1 table row of node id s (partition-major table layout)
    def l1row(s):
        return (s % 128) * NT + s // 128

    in_maps = []
    for r in range(cfg.ncores):
        # merged partition-major one-hot matrices: [128 part, nch, {Mm,Dm}, 128]
        MDT = np.zeros((128, nch, 2, 128), dtype=ml_dtypes.float8_e4m3fn)
        srcflat = np.zeros(nch * 128, dtype=np.int64)
        for wl in range(cfg.wpc):
            w = r * cfg.wpc + wl
            eids = order[bounds[w] : bounds[w + 1]]
            ne = len(eids)
            ck0 = cum[wl]
            j = np.arange(ne)
            ck = ck0 + j // 128
            sl = j % 128
            dloc = (dst[eids] - w * NW).astype(np.int64)
            # Mm[ck][dloc, sl] = 1 ; Mm[ck][NW:, sl] = edge_attr  (part dim first)
            MDT[dloc, ck, 0, sl] = 1.0
            MDT[NW:128, ck, 0, sl] = _f8(ea[eids]).T
            # Dm[ck][sl, dloc] = 1
            MDT[sl, ck, 1, dloc] = 1.0
            srcflat[ck0 * 128 + j] = src[eids]
        # gather index tiles: idx j of window w -> [j%16, off16+j//16], x8 replicate
        tot16 = nch * 8

        def build_idx(rows):
            idx16 = np.zeros((16, tot16), dtype=np.int16)
            for wl in range(cfg.wpc):
                o = cum[wl]
                nid = cw[wl] * 128
                sf = rows[o * 128 : o * 128 + nid]
                idx16[:, o * 8 : o * 8 + nid // 16] = sf.reshape(-1, 16).T
            return np.tile(idx16, (8, 1))

        srcidx1 = build_idx(l1row(srcflat).astype(np.int16))
        srcidx2 = build_idx(l2row(srcflat).astype(np.int16))

        # batch one-hot (mask for pooling): [wpc, 128, G]
        b1 = np.zeros((cfg.wpc, 128, cfg.g), dtype=ml_dtypes.bfloat16)
        nodes = np.arange(r * cfg.npc, (r + 1) * cfg.npc)
        valid = nodes < cfg.n
        bb = np.where(valid, batch[np.minimum(nodes, cfg.n - 1)], -1)
        for wl in range(cfg.wpc):
            seg = bb[wl * NW : (wl + 1) * NW]
            ii = np.where(seg >= 0)[0]
            b1[wl, ii, seg[ii]] = 1.0

        # own x slice, transposed (for xr1 projection)
        xs = np.zeros((cfg.npc, IN), dtype=np.float32)
        nv = int(valid.sum())
        xs[:nv] = np.asarray(x, dtype=np.float32)[nodes[:nv]]
        in_maps.append(
            {
                "xT": _bf(xs.T),
                "xTf": xTf,
                "MDT": MDT,
                "srcidx1": srcidx1,
                "srcidx2": srcidx2,
                "b1hot": b1,
            }
        )
    return in_maps


def make_consts(W1l, W1r, W1e, att1, W2l, W2r, W2e, att2, Wp, bp, Wv, bv):
    return {
        "w1l": _bf(W1l),
        "w1r": _bf(W1r),
        "w1e_t": _bf(np.tile(np.asarray(W1e)[:, None, :], (1, WPC, 1))),
        "att1bc": _bf(np.tile(np.asarray(att1).reshape(1, 1, -1), (128, 4, 1))),
        "w2l": _bf(W2l),
        "w2r": _bf(W2r),
        "w2e_t": _bf(np.tile(np.asarray(W2e)[:, None, :], (1, WPC, 1))),
        "att2bc": _bf(np.tile(np.asarray(att2).reshape(1, 1, -1), (128, 4, 1))),
        "i128bf": _bf(np.eye(128)),
        "i128f": _f32(np.eye(128)),
        "wpv": _f32(np.concatenate([_f32(Wp), _f32(Wv)], axis=1)),
        "bpv": _f32(
            np.tile(
                np.concatenate([_f32(bp), _f32(bv)]).reshape(1, ACT + 1), (G, 1)
            )
        ),
    }


# ================================================================ device side
def build_program(cfg: Cfg):
    import concourse.bass as bass
    import concourse.bacc as bacc
    import concourse.mybir as mybir
    import concourse.tile as tile
    from concourse import library_config
    from concourse.bass import broadcast_tensor_aps

    f32, bf16, i16 = mybir.dt.float32, mybir.dt.bfloat16, mybir.dt.int16
    f8 = mybir.dt.float8e4
    AF = mybir.ActivationFunctionType
    ALU = mybir.AluOpType

    wpc, nch, cwins = cfg.wpc, cfg.nch, cfg.cwins
    cwmax = cfg.cwmax
    npc, npad, g = cfg.npc, cfg.npad, cfg.g
    cum = np.concatenate([[0], np.cumsum(cwins)]).astype(int)
    lgdt = bf16 if LG_BF16 else f32

    nc = bacc.Bacc("TRN2", num_swdge_queues=4)

    # ---- I/O
    xT = nc.dram_tensor("xT", [IN, npc], bf16, kind="ExternalInput")
    xTf = nc.dram_tensor("xTf", [IN, NTAB], bf16, kind="ExternalInput")
    MDT = nc.dram_tensor("MDT", [128, nch, 2, 128], f8, kind="ExternalInput")
    srcidx1 = nc.dram_tensor("srcidx1", [128, nch * 8], i16, kind="ExternalInput")
    srcidx2 = nc.dram_tensor("srcidx2", [128, nch * 8], i16, kind="ExternalInput")
    b1hot = nc.dram_tensor("b1hot", [wpc, 128, g], bf16, kind="ExternalInput")
    w1l = nc.dram_tensor("w1l", [IN, HID], bf16, kind="ExternalInput")
    w1r = nc.dram_tensor("w1r", [IN, HID], bf16, kind="ExternalInput")
    w1e_t = nc.dram_tensor("w1e_t", [EDGE_D, wpc, HID], bf16, kind="ExternalInput")
    att1bc = nc.dram_tensor("att1bc", [128, 4, HID], bf16, kind="ExternalInput")
    w2l = nc.dram_tensor("w2l", [HID, HID], bf16, kind="ExternalInput")
    w2r = nc.dram_tensor("w2r", [HID, HID], bf16, kind="ExternalInput")
    w2e_t = nc.dram_tensor("w2e_t", [EDGE_D, wpc, HID], bf16, kind="ExternalInput")
    att2bc = nc.dram_tensor("att2bc", [128, 4, HID], bf16, kind="ExternalInput")
    i128bf = nc.dram_tensor("i128bf", [128, 128], bf16, kind="ExternalInput")
    i128f = nc.dram_tensor("i128f", [128, 128], f32, kind="ExternalInput")
    wpv = nc.dram_tensor("wpv", [HID, ACT + 1], f32, kind="ExternalInput")
    bpv = nc.dram_tensor("bpv", [g, ACT + 1], f32, kind="ExternalInput")
    out = nc.dram_tensor("out", [g, ACT + 1], f32, kind="ExternalOutput")

    # ---- internal DRAM (gather tables in fp8: halves gather traffic)
    xl1_full = nc.dram_tensor("xl1_full", [NTAB, HID], f8)
    xl2_slice = nc.dram_tensor("xl2_slice", [npc, HID], f8)
    xl2_full = nc.dram_tensor("xl2_full", [npad, HID], f8, addr_space="Shared")
    pp_in = nc.dram_tensor("pp_in", [g, HID + 1], f32)
    pp_out = nc.dram_tensor("pp_out", [g, HID + 1], f32, addr_space="Shared")

    taps = {}
    if cfg.debug_taps:
        taps["t_xl1full"] = nc.dram_tensor(
            "t_xl1full", [NTAB, HID], f8, kind="ExternalOutput"
        )
        taps["t_h1"] = nc.dram_tensor(
            "t_h1", [wpc, 128, HID], bf16, kind="ExternalOutput"
        )
        taps["t_pp"] = nc.dram_tensor(
            "t_pp", [g, HID + 1], f32, kind="ExternalOutput"
        )

    rg = [list(range(cfg.ncores))]

    with tile.TileContext(nc) as tc, ExitStack() as ctx:
        nc.gpsimd.load_library(library_config.mlp)

        consts = ctx.enter_context(tc.tile_pool(name="consts", bufs=1))
        # resident constants
        sb_att1 = consts.tile([128, 4, HID], bf16)
        nc.sync.dma_start(out=sb_att1, in_=att1bc[:, :, :])
        sb_att2 = consts.tile([128, 4, HID], bf16)
        nc.sync.dma_start(out=sb_att2, in_=att2bc[:, :, :])
        sb_ibf = consts.tile([128, 128], bf16)
        nc.sync.dma_start(out=sb_ibf, in_=i128bf[:, :])
        sb_if = consts.tile([128, 128], f32)
        nc.sync.dma_start(out=sb_if, in_=i128f[:, :])
        sb_w1l = consts.tile([IN, HID], bf16)
        nc.sync.dma_start(out=sb_w1l, in_=w1l[:, :])
        sb_w1r = consts.tile([IN, HID], bf16)
        nc.sync.dma_start(out=sb_w1r, in_=w1r[:, :])
        sb_w2l = consts.tile([128, 2, HID], bf16)
        nc.sync.dma_start(out=sb_w2l, in_=w2l.rearrange("(j p) f -> p j f", p=128))
        sb_w2r = consts.tile([128, 2, HID], bf16)
        nc.sync.dma_start(out=sb_w2r, in_=w2r.rearrange("(j p) f -> p j f", p=128))
        sb_idx1 = consts.tile([128, nch * 8], i16)
        nc.sync.dma_start(out=sb_idx1, in_=srcidx1[:, :])
        sb_idx2 = consts.tile([128, nch * 8], i16)
        nc.sync.dma_start(out=sb_idx2, in_=srcidx2[:, :])
        sb_b1h = consts.tile([128, wpc, g], bf16)
        nc.sync.dma_start(out=sb_b1h, in_=b1hot.rearrange("w p g -> p w g"))
        sb_wpv = consts.tile([128, 2, ACT + 1], f32)
        nc.sync.dma_start(out=sb_wpv, in_=wpv.rearrange("(j p) f -> p j f", p=128))
        sb_bpv = consts.tile([g, ACT + 1], f32)
        nc.sync.dma_start(out=sb_bpv, in_=bpv[:, :])
        # resident xr tables (rows 0:NW per window = xr, rows NW:128 = We)
        sb_xr1 = consts.tile([128, wpc, HID], bf16)
        nc.sync.dma_start(out=sb_xr1[NW:128, :, :], in_=w1e_t[:, :, :])
        sb_xr2 = consts.tile([128, wpc, HID], bf16)
        nc.sync.dma_start(out=sb_xr2[NW:128, :, :], in_=w2e_t[:, :, :])
        # xl2 staging tile
        sb_xl2st = consts.tile([128, wpc, HID], f8)

        # ---------------- P1: build xl1 gather table (full, local) + own xr1
        _sid, _ = nc.enter_named_scope("p1", False)
        with tc.tile_pool(name="p1", bufs=1) as p1, tc.tile_pool(
            name="p1st", bufs=4
        ) as p1st, tc.tile_pool(name="p1ps", bufs=4, space="PSUM") as p1ps:
            xt_all = p1.tile([IN, npc], bf16)
            nc.sync.dma_start(out=xt_all, in_=xT[:, :])
            xtf = p1.tile([IN, NTAB], bf16)
            nc.sync.dma_start(out=xtf, in_=xTf[:, :])
            for w in range(wpc):
                ps_r = p1ps.tile([128, HID], f32)
                nc.tensor.matmul(
                    ps_r[0:NW, :],
                    xt_all[:, w * NW : (w + 1) * NW],
                    sb_w1r,
                    start=True,
                    stop=True,
                )
                nc.vector.tensor_copy(sb_xr1[0:NW, w, :], ps_r[0:NW, :])
            # partition-major table: row of node n = (n%128)*NT + n//128
            tab_pm = xl1_full.rearrange("(p t) f -> p t f", p=128)
            for t0 in range(0, NT, TBATCH):
                tb = min(TBATCH, NT - t0)
                st = p1st.tile([128, TBATCH, HID], f8, tag="xlst")
                for ti in range(tb):
                    t = t0 + ti
                    ps_l = p1ps.tile([128, HID], f32)
                    nc.tensor.matmul(
                        ps_l, xtf[:, t * 128 : (t + 1) * 128], sb_w1l,
                        start=True, stop=True,
                    )
                    if t % 2 == 0:
                        nc.scalar.copy(st[:, ti, :], ps_l)
                    else:
                        nc.vector.tensor_copy(st[:, ti, :], ps_l)
                nc.sync.dma_start(
                    out=tab_pm[:, t0 : t0 + tb, :], in_=st[:, 0:tb, :]
                )
        nc.leave_named_scope("p1", _sid, False)
        if cfg.debug_taps:
            nc.sync.dma_start(out=taps["t_xl1full"][:, :], in_=xl1_full[:, :])

        _stops = ["p1", "l1", "l2", "full"]

        def _on(phase):
            return _stops.index(cfg.stop_after) >= _stops.index(phase)

        # ---------------- edge phase (shared for both layers)
        def edge_phase(layer):
            table = xl1_full if layer == 1 else xl2_full
            sb_idx = sb_idx1 if layer == 1 else sb_idx2
            xr_res = sb_xr1 if layer == 1 else sb_xr2
            attbc = sb_att1 if layer == 1 else sb_att2
            nheads = HEADS if layer == 1 else 1
            vw = HID + nheads  # V width / nd width
            lctx = ExitStack()
            ep = lctx.enter_context(tc.tile_pool(name=f"ep{layer}", bufs=5))
            sm = lctx.enter_context(tc.tile_pool(name=f"sm{layer}", bufs=5))
            ps_s = lctx.enter_context(
                tc.tile_pool(name=f"pss{layer}", bufs=3, space="PSUM")
            )
            ps_nd = lctx.enter_context(
                tc.tile_pool(name=f"psnd{layer}", bufs=2, space="PSUM")
            )
            if layer == 1:
                ps_pr = lctx.enter_context(
                    tc.tile_pool(name="pspr", bufs=1, space="PSUM")
                )
                ps_tp = lctx.enter_context(
                    tc.tile_pool(name="pstp", bufs=1, space="PSUM")
                )
                pool_ps = None
            else:
                ps_pool = lctx.enter_context(
                    tc.tile_pool(name="pspool", bufs=1, space="PSUM")
                )
                pool_ps = ps_pool.tile([g, HID + 1], f32)

            # pass A: gather + projections + logits + weighted values (vg)
            def pass_a(w):
                cw = cwins[w]
                ck0 = int(cum[w])
                xlg = ep.tile([128, cwmax, HID], f8, tag="xlg")
                nsplit = min(4, cw)
                gsz = [
                    cw // nsplit + (1 if i < cw % nsplit else 0)
                    for i in range(nsplit)
                ]
                go = 0
                for gi, gs in enumerate(gsz):
                    nc.gpsimd.dma_gather(
                        xlg[:, go : go + gs, :],
                        table[:, :],
                        sb_idx[:, (ck0 + go) * 8 : (ck0 + go + gs) * 8],
                        gs * 128,
                        gs * 128,
                        HID,
                        single_packet=False,
                        queue_num=gi,
                    )
                    go += gs
                mdt = ep.tile([128, cwmax, 2, 128], f8, tag="mdt")
                nc.sync.dma_start(
                    out=mdt[:, 0:cw, :, :], in_=MDT[:, ck0 : ck0 + cw, :, :]
                )
                msb = mdt[:, :, 0, :]
                dsb = mdt[:, :, 1, :]
                rw = xr_res[:, w, :]
                vg = sm.tile([128, cwmax, vw], bf16, tag="vg")

                for g0 in range(0, cw, 4):
                    gl = min(4, cw - g0)
                    m_g = sm.tile([128, 4, HID], bf16, tag="m")
                    for p0 in range(0, gl, 2):
                        pl = min(2, gl - p0)
                        ps = ps_s.tile([128, 2, HID], f32)
                        for ci in range(p0, p0 + pl):
                            c = g0 + ci
                            nc.tensor.matmul(
                                ps[:, ci - p0, :],
                                msb[:, c, :],
                                rw,
                                start=True,
                                stop=False,
                            )
                            nc.tensor.matmul(
                                ps[:, ci - p0, :],
                                sb_ibf,
                                xlg[:, c, :],
                                start=False,
                                stop=True,
                            )
                        nc.scalar.activation(
                            m_g[:, p0 : p0 + pl, :],
                            ps[:, 0:pl, :],
                            AF.Prelu,
                            alpha=NEG_SLOPE,
                        )
                    # batched attention logits over the chunk group
                    lgf = sm.tile([128, 4, nheads], lgdt if nheads == 4 else f32,
                                  tag="lg")
                    if nheads == 4:
                        wv_g = sm.tile([128, 4, HID], bf16, tag="wv")
                        nc.vector.tensor_mul(
                            wv_g[:, 0:gl, :], m_g[:, 0:gl, :], attbc[:, 0:gl, :]
                        )
                        with nc.allow_low_precision("logit reduce in bf16 ok"):
                            nc.vector.tensor_reduce(
                                lgf[:, 0:gl, :],
                                wv_g[:, 0:gl, :].rearrange(
                                    "p c (h k) -> p c h k", h=4
                                ),
                                axis=mybir.AxisListType.X,
                                op=ALU.add,
                            )
                    else:
                        wv_g = sm.tile([128, 4, HID], bf16, tag="wv")
                        nc.vector.tensor_mul(
                            wv_g[:, 0:gl, :], m_g[:, 0:gl, :], attbc[:, 0:gl, :]
                        )
                        nc.vector.tensor_reduce(
                            lgf[:, 0:gl, 0],
                            wv_g[:, 0:gl, :],
                            axis=mybir.AxisListType.X,
                            op=ALU.add,
                        )
                    expb = sm.tile([128, 4, nheads], bf16 if nheads == 4 else f32,
                                   tag="expb")
                    nc.scalar.activation(expb[:, 0:gl, :], lgf[:, 0:gl, :], AF.Exp)
                    if nheads == 4:
                        # v = xl[src] * p_h, all heads+chunks in one stride-0
                        # broadcast multiply
                        in0 = xlg[:, g0 : g0 + gl, :].rearrange(
                            "p c (h k) -> p c h k", h=4
                        )
                        in1 = expb[:, 0:gl, :].rearrange(
                            "p c (h k) -> p c h k", h=4
                        )
                        b0, b1 = broadcast_tensor_aps(in0, in1)
                        nc.vector.tensor_mul(
                            vg[:, g0 : g0 + gl, 0:HID].rearrange(
                                "p c (h k) -> p c h k", h=4
                            ),
                            b0,
                            b1,
                        )
                    else:
                        for ci in range(gl):
                            nc.vector.tensor_scalar_mul(
                                vg[:, g0 + ci, 0:HID],
                                xlg[:, g0 + ci, :],
                                expb[:, ci, 0:1],
                            )
                    nc.vector.tensor_copy(
                        vg[:, g0 : g0 + gl, HID:vw], expb[:, 0:gl, :]
                    )
                return dict(cw=cw, dsb=dsb, vg=vg)

            # pass B: scatter-accumulate nd + window epilogue
            def pass_b(w, st):
                cw, dsb, vg = st["cw"], st["dsb"], st["vg"]
                nd = ps_nd.tile([128, vw], f32)
                for c in range(cw):
                    nc.tensor.matmul(
                        nd,
                        dsb[:, c, :],
                        vg[:, c, :],
                        start=(c == 0),
                        stop=(c == cw - 1),
                    )
                dent = sm.tile([128, nheads], f32, tag="dent")
                nc.vector.tensor_scalar_add(dent, nd[:, HID:vw], 1e-16)
                rec = sm.tile([128, nheads], f32, tag="rec")
                nc.vector.reciprocal(rec, dent)
                if layer == 1:
                    hb = sm.tile([128, HID], bf16, tag="hb")
                else:
                    h2v = sm.tile([128, HID + 1], bf16, tag="h2v")
                    hb = h2v[:, 0:HID]
                for h in range(nheads):
                    sl = (
                        slice(h * 64, (h + 1) * 64)
                        if nheads == 4
                        else slice(0, HID)
                    )
                    nc.vector.tensor_scalar(
                        out=hb[:, sl],
                        in0=nd[:, sl],
                        scalar1=rec[:, h : h + 1],
                        scalar2=0.0,
                        op0=ALU.mult,
                        op1=ALU.max,
                    )
                if layer == 1:
                    if cfg.debug_taps:
                        nc.sync.dma_start(out=taps["t_h1"][w, :, :], in_=hb)
                    h1t = sm.tile([128, 2, 128], bf16, tag="h1t")
                    for j in range(2):
                        tp = ps_tp.tile([128, 128], bf16)
                        nc.tensor.transpose(
                            tp, hb[:, j * 128 : (j + 1) * 128], sb_ibf
                        )
                        nc.vector.tensor_copy(h1t[:, j, :], tp)
                    ps_xl2 = ps_pr.tile([128, HID], f32)
                    ps_xr2 = ps_pr.tile([128, HID], f32)
                    for j in range(2):
                        nc.tensor.matmul(
                            ps_xl2,
                            h1t[:, j, :],
                            sb_w2l[:, j, :],
                            start=(j == 0),
                            stop=(j == 1),
                        )
                        nc.tensor.matmul(
                            ps_xr2,
                            h1t[:, j, :],
                            sb_w2r[:, j, :],
                            start=(j == 0),
                            stop=(j == 1),
                        )
                    nc.scalar.copy(sb_xl2st[0:NW, w, :], ps_xl2[0:NW, :])
                    nc.vector.tensor_copy(sb_xr2[0:NW, w, :], ps_xr2[0:NW, :])
                else:
                    nc.vector.memset(h2v[:, HID : HID + 1], 1.0)
                    if os.environ.get("KERNEL_NO_POOL", "0") != "1":
                        nc.tensor.matmul(
                            pool_ps,
                            sb_b1h[:, w, :],
                            h2v,
                            start=(w == 0),
                            stop=(w == wpc - 1),
                        )

            # after the last pass_b of an ag2 piece (layer 1 only): store the
            # piece's xl2 rows and launch its partial AllGather
            def ag2_piece(p):
                w0, w1 = PB[p], PB[p + 1]
                nc.sync.dma_start(
                    out=xl2_slice.rearrange("(w q) f -> q w f", q=NW)[:, w0:w1, :],
                    in_=sb_xl2st[0:NW, w0:w1, :],
                )
                o0 = cfg.ncores * NW * w0
                o1 = cfg.ncores * NW * w1
                nc.gpsimd.collective_compute(
                    "AllGather",
                    mybir.AluOpType.bypass,
                    ins=[xl2_slice[w0 * NW : w1 * NW, :]],
                    outs=[xl2_full[o0:o1, :]],
                    replica_groups=rg,
                )

            # software pipeline: B(w-1) is emitted after A(w) so the PE can
            # run window w's projection matmuls while window w-1's vg chain
            # finishes on the DVE
            prev = None
            for w in range(wpc):
                st = pass_a(w)
                if prev is not None:
                    pass_b(w - 1, prev)
                    if layer == 1 and w in PB[1:-1]:
                        ag2_piece(PB.index(w) - 1)
                prev = st
            pass_b(wpc - 1, prev)
            if layer == 1:
                ag2_piece(PIECES - 1)

            if layer == 2:
                pps = sm.tile([g, HID + 1], f32, tag="pps")
                nc.scalar.copy(pps, pool_ps)
                nc.sync.dma_start(out=pp_in[:, :], in_=pps)
            lctx.close()

        if _on("l1"):
            _sid, _ = nc.enter_named_scope("l1", False)
            edge_phase(1)
            nc.leave_named_scope("l1", _sid, False)

        if _on("l2"):
            _sid, _ = nc.enter_named_scope("l2", False)
            edge_phase(2)
            nc.leave_named_scope("l2", _sid, False)

        if not _on("full"):
            with tc.tile_pool(name="dummy", bufs=1) as dp:
                ob = dp.tile([g, ACT + 1], f32)
                nc.vector.memset(ob, 0.0)
                nc.sync.dma_start(out=out[:, :], in_=ob)
            nc_done = True
        else:
            nc_done = False

        # ---------------- P6: exchange pool partials (AllToAll with the
        # partial replicated into every block = direct-send AllGather; the
        # ring AllReduce costs ~80us for 65KB)
        if not nc_done:
            _sid, _ = nc.enter_named_scope("ar", False)
            nc.gpsimd.collective_compute(
                "AllReduce",
                mybir.AluOpType.add,
                ins=[pp_in[:, :]],
                outs=[pp_out[:, :]],
                replica_groups=rg,
            )
            nc.leave_named_scope("ar", _sid, False)

        # ---------------- P7: head (replicated)
        if not nc_done:
          with tc.tile_pool(name="hd", bufs=1) as hd, tc.tile_pool(
            name="hdps", bufs=2, space="PSUM"
          ) as hdps:
            pp = hd.tile([g, HID + 1], f32)
            nc.sync.dma_start(out=pp, in_=pp_out[:, :])
            cnt = hd.tile([g, 1], f32)
            nc.vector.tensor_scalar_max(cnt, pp[:, HID : HID + 1], 1.0)
            rcnt = hd.tile([g, 1], f32)
            nc.vector.reciprocal(rcnt, cnt)
            gt = hd.tile([g, HID], f32)
            nc.vector.tensor_scalar_mul(gt, pp[:, 0:HID], rcnt[:, 0:1])
            gT = hd.tile([128, 2, g], f32)
            for j in range(2):
                tp = hdps.tile([128, g], f32)
                nc.tensor.transpose(
                    tp, gt[:, j * 128 : (j + 1) * 128], sb_if[0:g, 0:g]
                )
                nc.vector.tensor_copy(gT[:, j, :], tp)
            ps_o = hdps.tile([g, ACT + 1], f32)
            for j in range(2):
                nc.tensor.matmul(
                    ps_o, gT[:, j, :], sb_wpv[:, j, :], start=(j == 0), stop=(j == 1)
                )
            ob = hd.tile([g, ACT + 1], f32)
            nc.vector.tensor_add(ob, ps_o, sb_bpv)
            nc.sync.dma_start(out=out[:, :], in_=ob)

    nc.finalize()
    return nc


# ================================================================ entry point
_CACHE = {}


def kernel(
    x,
    edge_index,
    batch,
    edge_attr,
    W1l,
    W1r,
    W1e,
    att1,
    b1,
    W2l,
    W2r,
    W2e,
    att2,
    b2,
    Wp,
    bp,
    Wv,
    bv,
    _trace=False,
):
    from concourse.bass_utils import run_bass_kernel_spmd

    cfg = Cfg(stop_after=os.environ.get("STOP_AFTER", "full"))
    in_maps = preprocess(cfg, x, edge_index, batch, edge_attr)
    consts = make_consts(W1l, W1r, W1e, att1, W2l, W2r, W2e, att2, Wp, bp, Wv, bv)
    for m in in_maps:
        m.update(consts)

    key = (tuple(cfg.cwins), cfg.stop_after)
    if key not in _CACHE:
        _CACHE[key] = build_program(cfg)
    nc = _CACHE[key]

    tc_env = os.environ.get("TRACE_CORES", "")
    extra = {}
    if _trace and tc_env:
        extra["trace_cores"] = [int(c) for c in tc_env.split(",")]
    res = run_bass_kernel_spmd(
        nc, in_maps, list(range(cfg.ncores)), trace=_trace, **extra
    )
    out = np.asarray(res.results[0]["out"], dtype=np.float32)
    if _trace:
        kernel.last_exec_time_ns = res.exec_time_ns
        kernel.last_results = res
    return out


# revision 43
# speedup vs baseline: 1.0011x; 1.0011x over previous
"""Trainium2 Bass kernel for nn_ActorCriticGAT (2-layer GATv2 + global-mean-pool
actor-critic head), distributed over 8 NeuronCores.

Optimizations over the v2 baseline (1134us -> ~930us):
 - no ag1 collective: every core computes the full xl1 gather table locally
   from the replicated x input (162 tiles of x @ W1l, partition-major table
   layout so the 27 batched DRAM writes are contiguous per partition).
 - ag2 pipelined: xl2 is AllGathered in 3 pieces (7 windows each) issued as
   soon as each piece's windows finish layer-1; xl2_full uses a piece-major
   row layout and the layer-2 gather indices are remapped host-side.
 - gather tables in fp8e4 (halves gather DMA + collective bytes; ~3% value
   quantization washes out through softmax normalization + mean-pool).
 - Mm/Dm merged into one partition-major fp8 DRAM tensor (one contiguous
   load per window; one-hot entries exact in fp8).
 - gathers stay 4-way split across the 4 SWDGE queues (single merged
   gathers serialize their transfer and cost 2x on the GpSimd engine).
 - exp activation writes bf16 directly (no f32 CAST pass on the DVE).
 - final pool AllReduce kept minimal (65KB); its ~77us cost is core-skew
   bound (AllGather/AllToAll variants measured identical).

Sharding: nodes are padded to 8*21*123 = 20664 and split into 128-edge chunks
grouped by 123-node destination windows; each core owns 21 windows (2583 dst
nodes) and all edges pointing into them.  Per-edge work is edge-major
(edges on the SBUF partition axis):

  s    = xl1[src] + xr1[dst] + edge_attr @ W1e        (PSUM, 2 matmuls)
  m    = leaky_relu(s, 0.2)                           (ScalarE Prelu)
  lg   = att . m  (per head)                          (DVE mul+reduce)
  p    = exp(lg)  (softmax without max-subtraction: logits are O(1))
  num  = sum_e p * xl1[src],  den = sum_e p           (PE one-hot scatter matmul)
  h    = relu(num / den)
"""

import os
import sys

sys.path.insert(0, "/opt/trn_rl_repo")

import numpy as np
import ml_dtypes
from contextlib import ExitStack
from dataclasses import dataclass, field

# ---------------------------------------------------------------- problem dims
N, E, G = 20000, 320000, 64
IN, HID, HEADS, EDGE_D, ACT = 128, 256, 4, 5, 8
NEG_SLOPE = 0.2

NCORES = 8
NW = 128 - EDGE_D  # 123 dst nodes per window (+5 rows of edge_attr = K=128)
WPC = 21  # windows per core
NPC = NW * WPC  # 2583 nodes per core
NPAD = NCORES * NPC  # 20664 padded nodes
NT = (NPAD + 127) // 128  # 162 xl1-table tiles
NTAB = NT * 128  # 20736 rows in the xl1 gather table
PB = [0, 7, 14, 21]  # ag2 piece boundaries (windows)
PIECES = len(PB) - 1
TBATCH = 6  # xl1-table tiles per batched DRAM write (162 = 27 batches)
LG_BF16 = os.environ.get("KERNEL_LG_F32", "0") != "1"


@dataclass
class Cfg:
    n: int = N
    e: int = E
    g: int = G
    ncores: int = NCORES
    wpc: int = WPC
    cwins: list = field(default_factory=list)  # chunks per window slot [wpc]
    debug_taps: bool = False
    stop_after: str = "full"  # p1 | l1 | l2 | full

    @property
    def npc(self):
        return NW * self.wpc

    @property
    def npad(self):
        return self.ncores * self.npc

    @property
    def nch(self):
        return sum(self.cwins)

    @property
    def cwmax(self):
        return max(self.cwins)


def _bf(x):
    return np.asarray(x, dtype=ml_dtypes.bfloat16)


def _f8(x):
    return np.asarray(x, dtype=ml_dtypes.float8_e4m3fn)


def _f32(x):
    return np.ascontiguousarray(x, dtype=np.float32)


# ================================================================ host side
def preprocess(cfg: Cfg, x, edge_index, batch, edge_attr):
    """Build per-core input maps. Returns in_maps (cfg.cwins filled)."""
    src = np.asarray(edge_index[0], dtype=np.int64)
    dst = np.asarray(edge_index[1], dtype=np.int64)
    ea = _f32(edge_attr)
    batch = np.asarray(batch, dtype=np.int64)

    nwin_total = cfg.ncores * cfg.wpc
    win = dst // NW  # window id of each edge
    order = np.argsort(win, kind="stable")
    win_sorted = win[order]
    bounds = np.searchsorted(win_sorted, np.arange(nwin_total + 1))

    # chunks per window slot (equal across cores for SPMD)
    cw = np.zeros(cfg.wpc, dtype=np.int64)
    for w in range(nwin_total):
        cnt = bounds[w + 1] - bounds[w]
        slot = w % cfg.wpc
        cw[slot] = max(cw[slot], (cnt + 127) // 128)
    cw = np.maximum(cw, 1)
    cfg.cwins = [int(c) for c in cw]

    nch = cfg.nch
    cum = np.concatenate([[0], np.cumsum(cw)])  # chunk offset of each window slot

    # layer-2 table row of global node id s (piece-major layout)
    pbv = np.asarray(PB)

    def l2row(s):
        r = s // cfg.npc
        rem = s % cfg.npc
        w = rem // NW
        i = rem % NW
        p = np.searchsorted(pbv, w, side="right") - 1
        wp = pbv[p + 1] - pbv[p]  # windows in piece p
        return (
            cfg.ncores * NW * pbv[p]
            + r * (wp * NW)
            + (w - pbv[p]) * NW
            + i
        )

    # full padded x^T (replicated on every core)
    xf = np.zeros((NTAB, IN), dtype=np.float32)
    xf[: cfg.n] = np.asarray(x, dtype=np.float32)
    xTf = _bf(xf.T)

    # layer-1 table row of node id s (partition-major table layout)
    def l1row(s):
        return (s % 128) * NT + s // 128

    in_maps = []
    for r in range(cfg.ncores):
        # merged partition-major one-hot matrices: [128 part, nch, {Mm,Dm}, 128]
        MDT = np.zeros((128, nch, 2, 128), dtype=ml_dtypes.float8_e4m3fn)
        srcflat = np.zeros(nch * 128, dtype=np.int64)
        for wl in range(cfg.wpc):
            w = r * cfg.wpc + wl
            eids = order[bounds[w] : bounds[w + 1]]
            ne = len(eids)
            ck0 = cum[wl]
            j = np.arange(ne)
            ck = ck0 + j // 128
            sl = j % 128
            dloc = (dst[eids] - w * NW).astype(np.int64)
            # Mm[ck][dloc, sl] = 1 ; Mm[ck][NW:, sl] = edge_attr  (part dim first)
            MDT[dloc, ck, 0, sl] = 1.0
            MDT[NW:128, ck, 0, sl] = _f8(ea[eids]).T
            # Dm[ck][sl, dloc] = 1
            MDT[sl, ck, 1, dloc] = 1.0
            srcflat[ck0 * 128 + j] = src[eids]
        # gather index tiles: idx j of window w -> [j%16, off16+j//16], x8 replicate
        tot16 = nch * 8

        def build_idx(rows):
            idx16 = np.zeros((16, tot16), dtype=np.int16)
            for wl in range(cfg.wpc):
                o = cum[wl]
                nid = cw[wl] * 128
                sf = rows[o * 128 : o * 128 + nid]
                idx16[:, o * 8 : o * 8 + nid // 16] = sf.reshape(-1, 16).T
            return np.tile(idx16, (8, 1))

        srcidx1 = build_idx(l1row(srcflat).astype(np.int16))
        srcidx2 = build_idx(l2row(srcflat).astype(np.int16))

        # batch one-hot (mask for pooling): [wpc, 128, G]
        b1 = np.zeros((cfg.wpc, 128, cfg.g), dtype=ml_dtypes.bfloat16)
        nodes = np.arange(r * cfg.npc, (r + 1) * cfg.npc)
        valid = nodes < cfg.n
        bb = np.where(valid, batch[np.minimum(nodes, cfg.n - 1)], -1)
        for wl in range(cfg.wpc):
            seg = bb[wl * NW : (wl + 1) * NW]
            ii = np.where(seg >= 0)[0]
            b1[wl, ii, seg[ii]] = 1.0

        # own x slice, transposed (for xr1 projection)
        xs = np.zeros((cfg.npc, IN), dtype=np.float32)
        nv = int(valid.sum())
        xs[:nv] = np.asarray(x, dtype=np.float32)[nodes[:nv]]
        in_maps.append(
            {
                "xT": _bf(xs.T),
                "xTf": xTf,
                "MDT": MDT,
                "srcidx1": srcidx1,
                "srcidx2": srcidx2,
                "b1hot": b1,
            }
        )
    return in_maps


def make_consts(W1l, W1r, W1e, att1, W2l, W2r, W2e, att2, Wp, bp, Wv, bv):
    return {
        "w1l": _bf(W1l),
        "w1r": _bf(W1r),
        "w1e_t": _bf(np.tile(np.asarray(W1e)[:, None, :], (1, WPC, 1))),
        "att1bc": _bf(np.tile(np.asarray(att1).reshape(1, 1, -1), (128, 4, 1))),
        "w2l": _bf(W2l),
        "w2r": _bf(W2r),
        "w2e_t": _bf(np.tile(np.asarray(W2e)[:, None, :], (1, WPC, 1))),
        "att2bc": _bf(np.tile(np.asarray(att2).reshape(1, 1, -1), (128, 4, 1))),
        "i128bf": _bf(np.eye(128)),
        "i128f": _f32(np.eye(128)),
        "wpv": _f32(np.concatenate([_f32(Wp), _f32(Wv)], axis=1)),
        "bpv": _f32(
            np.tile(
                np.concatenate([_f32(bp), _f32(bv)]).reshape(1, ACT + 1), (G, 1)
            )
        ),
    }


# ================================================================ device side
def build_program(cfg: Cfg):
    import concourse.bass as bass
    import concourse.bacc as bacc
    import concourse.mybir as mybir
    import concourse.tile as tile
    from concourse import library_config
    from concourse.bass import broadcast_tensor_aps

    f32, bf16, i16 = mybir.dt.float32, mybir.dt.bfloat16, mybir.dt.int16
    f8 = mybir.dt.float8e4
    AF = mybir.ActivationFunctionType
    ALU = mybir.AluOpType

    wpc, nch, cwins = cfg.wpc, cfg.nch, cfg.cwins
    cwmax = cfg.cwmax
    npc, npad, g = cfg.npc, cfg.npad, cfg.g
    cum = np.concatenate([[0], np.cumsum(cwins)]).astype(int)
    lgdt = bf16 if LG_BF16 else f32

    nc = bacc.Bacc("TRN2", num_swdge_queues=4)

    # ---- I/O
    xT = nc.dram_tensor("xT", [IN, npc], bf16, kind="ExternalInput")
    xTf = nc.dram_tensor("xTf", [IN, NTAB], bf16, kind="ExternalInput")
    MDT = nc.dram_tensor("MDT", [128, nch, 2, 128], f8, kind="ExternalInput")
    srcidx1 = nc.dram_tensor("srcidx1", [128, nch * 8], i16, kind="ExternalInput")
    srcidx2 = nc.dram_tensor("srcidx2", [128, nch * 8], i16, kind="ExternalInput")
    b1hot = nc.dram_tensor("b1hot", [wpc, 128, g], bf16, kind="ExternalInput")
    w1l = nc.dram_tensor("w1l", [IN, HID], bf16, kind="ExternalInput")
    w1r = nc.dram_tensor("w1r", [IN, HID], bf16, kind="ExternalInput")
    w1e_t = nc.dram_tensor("w1e_t", [EDGE_D, wpc, HID], bf16, kind="ExternalInput")
    att1bc = nc.dram_tensor("att1bc", [128, 4, HID], bf16, kind="ExternalInput")
    w2l = nc.dram_tensor("w2l", [HID, HID], bf16, kind="ExternalInput")
    w2r = nc.dram_tensor("w2r", [HID, HID], bf16, kind="ExternalInput")
    w2e_t = nc.dram_tensor("w2e_t", [EDGE_D, wpc, HID], bf16, kind="ExternalInput")
    att2bc = nc.dram_tensor("att2bc", [128, 4, HID], bf16, kind="ExternalInput")
    i128bf = nc.dram_tensor("i128bf", [128, 128], bf16, kind="ExternalInput")
    i128f = nc.dram_tensor("i128f", [128, 128], f32, kind="ExternalInput")
    wpv = nc.dram_tensor("wpv", [HID, ACT + 1], f32, kind="ExternalInput")
    bpv = nc.dram_tensor("bpv", [g, ACT + 1], f32, kind="ExternalInput")
    out = nc.dram_tensor("out", [g, ACT + 1], f32, kind="ExternalOutput")

    # ---- internal DRAM (gather tables in fp8: halves gather traffic)
    xl1_full = nc.dram_tensor("xl1_full", [NTAB, HID], f8)
    xl2_slice = nc.dram_tensor("xl2_slice", [npc, HID], f8)
    xl2_full = nc.dram_tensor("xl2_full", [npad, HID], f8, addr_space="Shared")
    pp_in = nc.dram_tensor("pp_in", [g, HID + 1], f32)
    pp_out = nc.dram_tensor("pp_out", [g, HID + 1], f32, addr_space="Shared")

    taps = {}
    if cfg.debug_taps:
        taps["t_xl1full"] = nc.dram_tensor(
            "t_xl1full", [NTAB, HID], f8, kind="ExternalOutput"
        )
        taps["t_h1"] = nc.dram_tensor(
            "t_h1", [wpc, 128, HID], bf16, kind="ExternalOutput"
        )
        taps["t_pp"] = nc.dram_tensor(
            "t_pp", [g, HID + 1], f32, kind="ExternalOutput"
        )

    rg = [list(range(cfg.ncores))]

    with tile.TileContext(nc) as tc, ExitStack() as ctx:
        nc.gpsimd.load_library(library_config.mlp)

        consts = ctx.enter_context(tc.tile_pool(name="consts", bufs=1))
        # resident constants
        sb_att1 = consts.tile([128, 4, HID], bf16)
        nc.sync.dma_start(out=sb_att1, in_=att1bc[:, :, :])
        sb_att2 = consts.tile([128, 4, HID], bf16)
        nc.sync.dma_start(out=sb_att2, in_=att2bc[:, :, :])
        sb_ibf = consts.tile([128, 128], bf16)
        nc.sync.dma_start(out=sb_ibf, in_=i128bf[:, :])
        sb_if = consts.tile([128, 128], f32)
        nc.sync.dma_start(out=sb_if, in_=i128f[:, :])
        sb_w1l = consts.tile([IN, HID], bf16)
        nc.sync.dma_start(out=sb_w1l, in_=w1l[:, :])
        sb_w1r = consts.tile([IN, HID], bf16)
        nc.sync.dma_start(out=sb_w1r, in_=w1r[:, :])
        sb_w2l = consts.tile([128, 2, HID], bf16)
        nc.sync.dma_start(out=sb_w2l, in_=w2l.rearrange("(j p) f -> p j f", p=128))
        sb_w2r = consts.tile([128, 2, HID], bf16)
        nc.sync.dma_start(out=sb_w2r, in_=w2r.rearrange("(j p) f -> p j f", p=128))
        sb_idx1 = consts.tile([128, nch * 8], i16)
        nc.sync.dma_start(out=sb_idx1, in_=srcidx1[:, :])
        sb_idx2 = consts.tile([128, nch * 8], i16)
        nc.sync.dma_start(out=sb_idx2, in_=srcidx2[:, :])
        sb_b1h = consts.tile([128, wpc, g], bf16)
        nc.sync.dma_start(out=sb_b1h, in_=b1hot.rearrange("w p g -> p w g"))
        sb_wpv = consts.tile([128, 2, ACT + 1], f32)
        nc.sync.dma_start(out=sb_wpv, in_=wpv.rearrange("(j p) f -> p j f", p=128))
        sb_bpv = consts.tile([g, ACT + 1], f32)
        nc.sync.dma_start(out=sb_bpv, in_=bpv[:, :])
        # resident xr tables (rows 0:NW per window = xr, rows NW:128 = We)
        sb_xr1 = consts.tile([128, wpc, HID], bf16)
        nc.sync.dma_start(out=sb_xr1[NW:128, :, :], in_=w1e_t[:, :, :])
        sb_xr2 = consts.tile([128, wpc, HID], bf16)
        nc.sync.dma_start(out=sb_xr2[NW:128, :, :], in_=w2e_t[:, :, :])
        # xl2 staging tile
        sb_xl2st = consts.tile([128, wpc, HID], f8)

        # ---------------- P1: build xl1 gather table (full, local) + own xr1
        _sid, _ = nc.enter_named_scope("p1", False)
        with tc.tile_pool(name="p1", bufs=1) as p1, tc.tile_pool(
            name="p1st", bufs=4
        ) as p1st, tc.tile_pool(name="p1ps", bufs=4, space="PSUM") as p1ps:
            xt_all = p1.tile([IN, npc], bf16)
            nc.sync.dma_start(out=xt_all, in_=xT[:, :])
            xtf = p1.tile([IN, NTAB], bf16)
            nc.sync.dma_start(out=xtf, in_=xTf[:, :])
            for w in range(wpc):
                ps_r = p1ps.tile([128, HID], f32)
                nc.tensor.matmul(
                    ps_r[0:NW, :],
                    xt_all[:, w * NW : (w + 1) * NW],
                    sb_w1r,
                    start=True,
                    stop=True,
                )
                nc.vector.tensor_copy(sb_xr1[0:NW, w, :], ps_r[0:NW, :])
            # partition-major table: row of node n = (n%128)*NT + n//128
            tab_pm = xl1_full.rearrange("(p t) f -> p t f", p=128)
            for t0 in range(0, NT, TBATCH):
                tb = min(TBATCH, NT - t0)
                st = p1st.tile([128, TBATCH, HID], f8, tag="xlst")
                for ti in range(tb):
                    t = t0 + ti
                    ps_l = p1ps.tile([128, HID], f32)
                    nc.tensor.matmul(
                        ps_l, xtf[:, t * 128 : (t + 1) * 128], sb_w1l,
                        start=True, stop=True,
                    )
                    if t % 2 == 0:
                        nc.scalar.copy(st[:, ti, :], ps_l)
                    else:
                        nc.vector.tensor_copy(st[:, ti, :], ps_l)
                nc.sync.dma_start(
                    out=tab_pm[:, t0 : t0 + tb, :], in_=st[:, 0:tb, :]
                )
        nc.leave_named_scope("p1", _sid, False)
        if cfg.debug_taps:
            nc.sync.dma_start(out=taps["t_xl1full"][:, :], in_=xl1_full[:, :])

        _stops = ["p1", "l1", "l2", "full"]

        def _on(phase):
            return _stops.index(cfg.stop_after) >= _stops.index(phase)

        # ---------------- edge phase (shared for both layers)
        def edge_phase(layer):
            table = xl1_full if layer == 1 else xl2_full
            sb_idx = sb_idx1 if layer == 1 else sb_idx2
            xr_res = sb_xr1 if layer == 1 else sb_xr2
            attbc = sb_att1 if layer == 1 else sb_att2
            nheads = HEADS if layer == 1 else 1
            vw = HID + nheads  # V width / nd width
            lctx = ExitStack()
            ep = lctx.enter_context(tc.tile_pool(name=f"ep{layer}", bufs=5))
            sm = lctx.enter_context(tc.tile_pool(name=f"sm{layer}", bufs=5))
            ps_s = lctx.enter_context(
                tc.tile_pool(name=f"pss{layer}", bufs=3, space="PSUM")
            )
            ps_nd = lctx.enter_context(
                tc.tile_pool(name=f"psnd{layer}", bufs=2, space="PSUM")
            )
            if layer == 1:
                ps_pr = lctx.enter_context(
                    tc.tile_pool(name="pspr", bufs=1, space="PSUM")
                )
                ps_tp = lctx.enter_context(
                    tc.tile_pool(name="pstp", bufs=1, space="PSUM")
                )
                pool_ps = None
            else:
                ps_pool = lctx.enter_context(
                    tc.tile_pool(name="pspool", bufs=1, space="PSUM")
                )
                pool_ps = ps_pool.tile([g, HID + 1], f32)

            # pass A: gather + projections + logits + weighted values (vg)
            def pass_a(w):
                cw = cwins[w]
                ck0 = int(cum[w])
                xlg = ep.tile([128, cwmax, HID], f8, tag="xlg")
                nsplit = min(4, cw)
                gsz = [
                    cw // nsplit + (1 if i < cw % nsplit else 0)
                    for i in range(nsplit)
                ]
                go = 0
                for gi, gs in enumerate(gsz):
                    nc.gpsimd.dma_gather(
                        xlg[:, go : go + gs, :],
                        table[:, :],
                        sb_idx[:, (ck0 + go) * 8 : (ck0 + go + gs) * 8],
                        gs * 128,
                        gs * 128,
                        HID,
                        single_packet=False,
                        queue_num=gi,
                    )
                    go += gs
                mdt = ep.tile([128, cwmax, 2, 128], f8, tag="mdt")
                nc.sync.dma_start(
                    out=mdt[:, 0:cw, :, :], in_=MDT[:, ck0 : ck0 + cw, :, :]
                )
                msb = mdt[:, :, 0, :]
                dsb = mdt[:, :, 1, :]
                rw = xr_res[:, w, :]
                vg = sm.tile([128, cwmax, vw], bf16, tag="vg")

                for g0 in range(0, cw, 4):
                    gl = min(4, cw - g0)
                    m_g = sm.tile([128, 4, HID], bf16, tag="m")
                    for p0 in range(0, gl, 2):
                        pl = min(2, gl - p0)
                        ps = ps_s.tile([128, 2, HID], f32)
                        for ci in range(p0, p0 + pl):
                            c = g0 + ci
                            nc.tensor.matmul(
                                ps[:, ci - p0, :],
                                msb[:, c, :],
                                rw,
                                start=True,
                                stop=False,
                            )
                            nc.tensor.matmul(
                                ps[:, ci - p0, :],
                                sb_ibf,
                                xlg[:, c, :],
                                start=False,
                                stop=True,
                            )
                        nc.scalar.activation(
                            m_g[:, p0 : p0 + pl, :],
                            ps[:, 0:pl, :],
                            AF.Prelu,
                            alpha=NEG_SLOPE,
                        )
                    # batched attention logits over the chunk group
                    lgf = sm.tile([128, 4, nheads], lgdt if nheads == 4 else f32,
                                  tag="lg")
                    if nheads == 4:
                        wv_g = sm.tile([128, 4, HID], bf16, tag="wv")
                        nc.vector.tensor_mul(
                            wv_g[:, 0:gl, :], m_g[:, 0:gl, :], attbc[:, 0:gl, :]
                        )
                        with nc.allow_low_precision("logit reduce in bf16 ok"):
                            nc.vector.tensor_reduce(
                                lgf[:, 0:gl, :],
                                wv_g[:, 0:gl, :].rearrange(
                                    "p c (h k) -> p c h k", h=4
                                ),
                                axis=mybir.AxisListType.X,
                                op=ALU.add,
                            )
                    else:
                        wv_g = sm.tile([128, 4, HID], bf16, tag="wv")
                        nc.vector.tensor_mul(
                            wv_g[:, 0:gl, :], m_g[:, 0:gl, :], attbc[:, 0:gl, :]
                        )
                        nc.vector.tensor_reduce(
                            lgf[:, 0:gl, 0],
                            wv_g[:, 0:gl, :],
                            axis=mybir.AxisListType.X,
                            op=ALU.add,
                        )
                    expb = sm.tile([128, 4, nheads], bf16 if nheads == 4 else f32,
                                   tag="expb")
                    nc.scalar.activation(expb[:, 0:gl, :], lgf[:, 0:gl, :], AF.Exp)
                    if nheads == 4:
                        # v = xl[src] * p_h, all heads+chunks in one stride-0
                        # broadcast multiply
                        in0 = xlg[:, g0 : g0 + gl, :].rearrange(
                            "p c (h k) -> p c h k", h=4
                        )
                        in1 = expb[:, 0:gl, :].rearrange(
                            "p c (h k) -> p c h k", h=4
                        )
                        b0, b1 = broadcast_tensor_aps(in0, in1)
                        nc.vector.tensor_mul(
                            vg[:, g0 : g0 + gl, 0:HID].rearrange(
                                "p c (h k) -> p c h k", h=4
                            ),
                            b0,
                            b1,
                        )
                    else:
                        for ci in range(gl):
                            nc.vector.tensor_scalar_mul(
                                vg[:, g0 + ci, 0:HID],
                                xlg[:, g0 + ci, :],
                                expb[:, ci, 0:1],
                            )
                    nc.vector.tensor_copy(
                        vg[:, g0 : g0 + gl, HID:vw], expb[:, 0:gl, :]
                    )
                return dict(cw=cw, dsb=dsb, vg=vg)

            # pass B: scatter-accumulate nd + window epilogue
            def pass_b(w, st):
                cw, dsb, vg = st["cw"], st["dsb"], st["vg"]
                nd = ps_nd.tile([128, vw], f32)
                for c in range(cw):
                    nc.tensor.matmul(
                        nd,
                        dsb[:, c, :],
                        vg[:, c, :],
                        start=(c == 0),
                        stop=(c == cw - 1),
                    )
                dent = sm.tile([128, nheads], f32, tag="dent")
                nc.vector.tensor_scalar_add(dent, nd[:, HID:vw], 1e-16)
                rec = sm.tile([128, nheads], f32, tag="rec")
                nc.vector.reciprocal(rec, dent)
                if layer == 1:
                    hb = sm.tile([128, HID], bf16, tag="hb")
                else:
                    h2v = sm.tile([128, HID + 1], bf16, tag="h2v")
                    hb = h2v[:, 0:HID]
                for h in range(nheads):
                    sl = (
                        slice(h * 64, (h + 1) * 64)
                        if nheads == 4
                        else slice(0, HID)
                    )
                    nc.vector.tensor_scalar(
                        out=hb[:, sl],
                        in0=nd[:, sl],
                        scalar1=rec[:, h : h + 1],
                        scalar2=0.0,
                        op0=ALU.mult,
                        op1=ALU.max,
                    )
                if layer == 1:
                    if cfg.debug_taps:
                        nc.sync.dma_start(out=taps["t_h1"][w, :, :], in_=hb)
                    h1t = sm.tile([128, 2, 128], bf16, tag="h1t")
                    for j in range(2):
                        tp = ps_tp.tile([128, 128], bf16)
                        nc.tensor.transpose(
                            tp, hb[:, j * 128 : (j + 1) * 128], sb_ibf
                        )
                        nc.vector.tensor_copy(h1t[:, j, :], tp)
                    ps_xl2 = ps_pr.tile([128, HID], f32)
                    ps_xr2 = ps_pr.tile([128, HID], f32)
                    for j in range(2):
                        nc.tensor.matmul(
                            ps_xl2,
                            h1t[:, j, :],
                            sb_w2l[:, j, :],
                            start=(j == 0),
                            stop=(j == 1),
                        )
                        nc.tensor.matmul(
                            ps_xr2,
                            h1t[:, j, :],
                            sb_w2r[:, j, :],
                            start=(j == 0),
                            stop=(j == 1),
                        )
                    nc.scalar.copy(sb_xl2st[0:NW, w, :], ps_xl2[0:NW, :])
                    nc.vector.tensor_copy(sb_xr2[0:NW, w, :], ps_xr2[0:NW, :])
                else:
                    nc.vector.memset(h2v[:, HID : HID + 1], 1.0)
                    if os.environ.get("KERNEL_NO_POOL", "0") != "1":
                        nc.tensor.matmul(
                            pool_ps,
                            sb_b1h[:, w, :],
                            h2v,
                            start=(w == 0),
                            stop=(w == wpc - 1),
                        )

            # after the last pass_b of an ag2 piece (layer 1 only): store the
            # piece's xl2 rows and launch its partial AllGather
            def ag2_piece(p):
                w0, w1 = PB[p], PB[p + 1]
                nc.sync.dma_start(
                    out=xl2_slice.rearrange("(w q) f -> q w f", q=NW)[:, w0:w1, :],
                    in_=sb_xl2st[0:NW, w0:w1, :],
                )
                o0 = cfg.ncores * NW * w0
                o1 = cfg.ncores * NW * w1
                nc.gpsimd.collective_compute(
                    "AllGather",
                    mybir.AluOpType.bypass,
                    ins=[xl2_slice[w0 * NW : w1 * NW, :]],
                    outs=[xl2_full[o0:o1, :]],
                    replica_groups=rg,
                )

            # software pipeline: B(w-1) is emitted after A(w) so the PE can
            # run window w's projection matmuls while window w-1's vg chain
            # finishes on the DVE
            prev = None
            for w in range(wpc):
                st = pass_a(w)
                if prev is not None:
                    pass_b(w - 1, prev)
                    if layer == 1 and w in PB[1:-1]:
                        ag2_piece(PB.index(w) - 1)
                prev = st
            pass_b(wpc - 1, prev)
            if layer == 1:
                ag2_piece(PIECES - 1)

            if layer == 2:
                pps = sm.tile([g, HID + 1], f32, tag="pps")
                nc.scalar.copy(pps, pool_ps)
                nc.sync.dma_start(out=pp_in[:, :], in_=pps)
            lctx.close()

        if _on("l1"):
            _sid, _ = nc.enter_named_scope("l1", False)
            edge_phase(1)
            nc.leave_named_scope("l1", _sid, False)

        if _on("l2"):
            _sid, _ = nc.enter_named_scope("l2", False)
            edge_phase(2)
            nc.leave_named_scope("l2", _sid, False)

        if not _on("full"):
            with tc.tile_pool(name="dummy", bufs=1) as dp:
                ob = dp.tile([g, ACT + 1], f32)
                nc.vector.memset(ob, 0.0)
                nc.sync.dma_start(out=out[:, :], in_=ob)
            nc_done = True
        else:
            nc_done = False

        # ---------------- P6: exchange pool partials (AllToAll with the
        # partial replicated into every block = direct-send AllGather; the
        # ring AllReduce costs ~80us for 65KB)
        if not nc_done:
            _sid, _ = nc.enter_named_scope("ar", False)
            nc.gpsimd.collective_compute(
                "AllReduce",
                mybir.AluOpType.add,
                ins=[pp_in[:, :]],
                outs=[pp_out[:, :]],
                replica_groups=rg,
            )
            nc.leave_named_scope("ar", _sid, False)

        # ---------------- P7: head (replicated)
        if not nc_done:
          with tc.tile_pool(name="hd", bufs=1) as hd, tc.tile_pool(
            name="hdps", bufs=2, space="PSUM"
          ) as hdps:
            pp = hd.tile([g, HID + 1], f32)
            nc.sync.dma_start(out=pp, in_=pp_out[:, :])
            cnt = hd.tile([g, 1], f32)
            nc.vector.tensor_scalar_max(cnt, pp[:, HID : HID + 1], 1.0)
            rcnt = hd.tile([g, 1], f32)
            nc.vector.reciprocal(rcnt, cnt)
            gt = hd.tile([g, HID], f32)
            nc.vector.tensor_scalar_mul(gt, pp[:, 0:HID], rcnt[:, 0:1])
            gT = hd.tile([128, 2, g], f32)
            for j in range(2):
                tp = hdps.tile([128, g], f32)
                nc.tensor.transpose(
                    tp, gt[:, j * 128 : (j + 1) * 128], sb_if[0:g, 0:g]
                )
                nc.vector.tensor_copy(gT[:, j, :], tp)
            ps_o = hdps.tile([g, ACT + 1], f32)
            for j in range(2):
                nc.tensor.matmul(
                    ps_o, gT[:, j, :], sb_wpv[:, j, :], start=(j == 0), stop=(j == 1)
                )
            ob = hd.tile([g, ACT + 1], f32)
            nc.vector.tensor_add(ob, ps_o, sb_bpv)
            nc.sync.dma_start(out=out[:, :], in_=ob)

    nc.finalize()
    return nc


# ================================================================ entry point
_CACHE = {}


def kernel(
    x,
    edge_index,
    batch,
    edge_attr,
    W1l,
    W1r,
    W1e,
    att1,
    b1,
    W2l,
    W2r,
    W2e,
    att2,
    b2,
    Wp,
    bp,
    Wv,
    bv,
    _trace=False,
):
    from concourse.bass_utils import run_bass_kernel_spmd

    cfg = Cfg(stop_after=os.environ.get("STOP_AFTER", "full"))
    in_maps = preprocess(cfg, x, edge_index, batch, edge_attr)
    consts = make_consts(W1l, W1r, W1e, att1, W2l, W2r, W2e, att2, Wp, bp, Wv, bv)
    for m in in_maps:
        m.update(consts)

    key = (tuple(cfg.cwins), cfg.stop_after)
    if key not in _CACHE:
        _CACHE[key] = build_program(cfg)
    nc = _CACHE[key]

    tc_env = os.environ.get("TRACE_CORES", "")
    extra = {}
    if _trace and tc_env:
        extra["trace_cores"] = [int(c) for c in tc_env.split(",")]
    res = run_bass_kernel_spmd(
        nc, in_maps, list(range(cfg.ncores)), trace=_trace, **extra
    )
    out = np.asarray(res.results[0]["out"], dtype=np.float32)
    if _trace:
        kernel.last_exec_time_ns = res.exec_time_ns
        kernel.last_results = res
    return out


# revision 44
# speedup vs baseline: 1.0473x; 1.0462x over previous
"""Trainium2 Bass kernel for nn_ActorCriticGAT (2-layer GATv2 + global-mean-pool
actor-critic head), distributed over 8 NeuronCores.

Optimizations over the v2 baseline (1134us -> ~930us):
 - no ag1 collective: every core computes the full xl1 gather table locally
   from the replicated x input (162 tiles of x @ W1l, partition-major table
   layout so the 27 batched DRAM writes are contiguous per partition).
 - ag2 pipelined: xl2 is AllGathered in 3 pieces (7 windows each) issued as
   soon as each piece's windows finish layer-1; xl2_full uses a piece-major
   row layout and the layer-2 gather indices are remapped host-side.
 - gather tables in fp8e4 (halves gather DMA + collective bytes; ~3% value
   quantization washes out through softmax normalization + mean-pool).
 - Mm/Dm merged into one partition-major fp8 DRAM tensor (one contiguous
   load per window; one-hot entries exact in fp8).
 - gathers stay 4-way split across the 4 SWDGE queues (single merged
   gathers serialize their transfer and cost 2x on the GpSimd engine).
 - exp activation writes bf16 directly (no f32 CAST pass on the DVE).
 - final pool AllReduce kept minimal (65KB); its ~77us cost is core-skew
   bound (AllGather/AllToAll variants measured identical).

Sharding: nodes are padded to 8*21*123 = 20664 and split into 128-edge chunks
grouped by 123-node destination windows; each core owns 21 windows (2583 dst
nodes) and all edges pointing into them.  Per-edge work is edge-major
(edges on the SBUF partition axis):

  s    = xl1[src] + xr1[dst] + edge_attr @ W1e        (PSUM, 2 matmuls)
  m    = leaky_relu(s, 0.2)                           (ScalarE Prelu)
  lg   = att . m  (per head)                          (DVE mul+reduce)
  p    = exp(lg)  (softmax without max-subtraction: logits are O(1))
  num  = sum_e p * xl1[src],  den = sum_e p           (PE one-hot scatter matmul)
  h    = relu(num / den)
"""

import os
import sys

sys.path.insert(0, "/opt/trn_rl_repo")

import numpy as np
import ml_dtypes
from contextlib import ExitStack
from dataclasses import dataclass, field

# ---------------------------------------------------------------- problem dims
N, E, G = 20000, 320000, 64
IN, HID, HEADS, EDGE_D, ACT = 128, 256, 4, 5, 8
NEG_SLOPE = 0.2

NCORES = 8
NW = 128 - EDGE_D  # 123 dst nodes per window (+5 rows of edge_attr = K=128)
WPC = 21  # windows per core
NPC = NW * WPC  # 2583 nodes per core
NPAD = NCORES * NPC  # 20664 padded nodes
NT = (NPAD + 127) // 128  # 162 xl1-table tiles
NTAB = NT * 128  # 20736 rows in the xl1 gather table
PB = [0, 8, 16, 21]  # ag2 piece boundaries (last piece smallest)
PIECES = len(PB) - 1
TBATCH = 6  # xl1-table tiles per batched DRAM write (162 = 27 batches)
LG_BF16 = os.environ.get("KERNEL_LG_F32", "0") != "1"


@dataclass
class Cfg:
    n: int = N
    e: int = E
    g: int = G
    ncores: int = NCORES
    wpc: int = WPC
    cwins: list = field(default_factory=list)  # chunks per window slot [wpc]
    debug_taps: bool = False
    stop_after: str = "full"  # p1 | l1 | l2 | full

    @property
    def npc(self):
        return NW * self.wpc

    @property
    def npad(self):
        return self.ncores * self.npc

    @property
    def nch(self):
        return sum(self.cwins)

    @property
    def cwmax(self):
        return max(self.cwins)


def _bf(x):
    return np.asarray(x, dtype=ml_dtypes.bfloat16)


def _f8(x):
    return np.asarray(x, dtype=ml_dtypes.float8_e4m3fn)


def _f32(x):
    return np.ascontiguousarray(x, dtype=np.float32)


# ================================================================ host side
def preprocess(cfg: Cfg, x, edge_index, batch, edge_attr):
    """Build per-core input maps. Returns in_maps (cfg.cwins filled)."""
    src = np.asarray(edge_index[0], dtype=np.int64)
    dst = np.asarray(edge_index[1], dtype=np.int64)
    ea = _f32(edge_attr)
    batch = np.asarray(batch, dtype=np.int64)

    nwin_total = cfg.ncores * cfg.wpc
    win = dst // NW  # window id of each edge
    order = np.argsort(win, kind="stable")
    win_sorted = win[order]
    bounds = np.searchsorted(win_sorted, np.arange(nwin_total + 1))

    # chunks per window slot (equal across cores for SPMD)
    cw = np.zeros(cfg.wpc, dtype=np.int64)
    for w in range(nwin_total):
        cnt = bounds[w + 1] - bounds[w]
        slot = w % cfg.wpc
        cw[slot] = max(cw[slot], (cnt + 127) // 128)
    cw = np.maximum(cw, 1)
    cfg.cwins = [int(c) for c in cw]

    nch = cfg.nch
    cum = np.concatenate([[0], np.cumsum(cw)])  # chunk offset of each window slot

    # layer-2 table row of global node id s (piece-major layout)
    pbv = np.asarray(PB)

    def l2row(s):
        r = s // cfg.npc
        rem = s % cfg.npc
        w = rem // NW
        i = rem % NW
        p = np.searchsorted(pbv, w, side="right") - 1
        wp = pbv[p + 1] - pbv[p]  # windows in piece p
        return (
            cfg.ncores * NW * pbv[p]
            + r * (wp * NW)
            + (w - pbv[p]) * NW
            + i
        )

    # full padded x^T (replicated on every core)
    xf = np.zeros((NTAB, IN), dtype=np.float32)
    xf[: cfg.n] = np.asarray(x, dtype=np.float32)
    xTf = _bf(xf.T)

    # layer-1 table row of node id s (partition-major table layout)
    def l1row(s):
        return (s % 128) * NT + s // 128

    in_maps = []
    for r in range(cfg.ncores):
        # merged partition-major one-hot matrices: [128 part, nch, {Mm,Dm}, 128]
        MDT = np.zeros((128, nch, 2, 128), dtype=ml_dtypes.float8_e4m3fn)
        srcflat = np.zeros(nch * 128, dtype=np.int64)
        for wl in range(cfg.wpc):
            w = r * cfg.wpc + wl
            eids = order[bounds[w] : bounds[w + 1]]
            ne = len(eids)
            ck0 = cum[wl]
            j = np.arange(ne)
            ck = ck0 + j // 128
            sl = j % 128
            dloc = (dst[eids] - w * NW).astype(np.int64)
            # Mm[ck][dloc, sl] = 1 ; Mm[ck][NW:, sl] = edge_attr  (part dim first)
            MDT[dloc, ck, 0, sl] = 1.0
            MDT[NW:128, ck, 0, sl] = _f8(ea[eids]).T
            # Dm[ck][sl, dloc] = 1
            MDT[sl, ck, 1, dloc] = 1.0
            srcflat[ck0 * 128 + j] = src[eids]
        # gather index tiles: idx j of window w -> [j%16, off16+j//16], x8 replicate
        tot16 = nch * 8

        def build_idx(rows):
            idx16 = np.zeros((16, tot16), dtype=np.int16)
            for wl in range(cfg.wpc):
                o = cum[wl]
                nid = cw[wl] * 128
                sf = rows[o * 128 : o * 128 + nid]
                idx16[:, o * 8 : o * 8 + nid // 16] = sf.reshape(-1, 16).T
            return np.tile(idx16, (8, 1))

        srcidx1 = build_idx(l1row(srcflat).astype(np.int16))
        srcidx2 = build_idx(l2row(srcflat).astype(np.int16))

        # batch one-hot (mask for pooling): [wpc, 128, G]
        b1 = np.zeros((cfg.wpc, 128, cfg.g), dtype=ml_dtypes.bfloat16)
        nodes = np.arange(r * cfg.npc, (r + 1) * cfg.npc)
        valid = nodes < cfg.n
        bb = np.where(valid, batch[np.minimum(nodes, cfg.n - 1)], -1)
        for wl in range(cfg.wpc):
            seg = bb[wl * NW : (wl + 1) * NW]
            ii = np.where(seg >= 0)[0]
            b1[wl, ii, seg[ii]] = 1.0

        # own x slice, transposed (for xr1 projection)
        xs = np.zeros((cfg.npc, IN), dtype=np.float32)
        nv = int(valid.sum())
        xs[:nv] = np.asarray(x, dtype=np.float32)[nodes[:nv]]
        in_maps.append(
            {
                "xT": _bf(xs.T),
                "xTf": xTf,
                "MDT": MDT,
                "srcidx1": srcidx1,
                "srcidx2": srcidx2,
                "b1hot": b1,
            }
        )
    return in_maps


def make_consts(W1l, W1r, W1e, att1, W2l, W2r, W2e, att2, Wp, bp, Wv, bv):
    return {
        "w1l": _bf(W1l),
        "w1r": _bf(W1r),
        "w1e_t": _bf(np.tile(np.asarray(W1e)[:, None, :], (1, WPC, 1))),
        "att1bc": _bf(np.tile(np.asarray(att1).reshape(1, 1, -1), (128, 4, 1))),
        "w2l": _bf(W2l),
        "w2r": _bf(W2r),
        "w2e_t": _bf(np.tile(np.asarray(W2e)[:, None, :], (1, WPC, 1))),
        "att2bc": _bf(np.tile(np.asarray(att2).reshape(1, 1, -1), (128, 4, 1))),
        "i128bf": _bf(np.eye(128)),
        "i128f": _f32(np.eye(128)),
        "wpv": _f32(np.concatenate([_f32(Wp), _f32(Wv)], axis=1)),
        "bpv": _f32(
            np.tile(
                np.concatenate([_f32(bp), _f32(bv)]).reshape(1, ACT + 1), (G, 1)
            )
        ),
    }


# ================================================================ device side
def build_program(cfg: Cfg):
    import concourse.bass as bass
    import concourse.bacc as bacc
    import concourse.mybir as mybir
    import concourse.tile as tile
    from concourse import library_config
    from concourse.bass import broadcast_tensor_aps

    f32, bf16, i16 = mybir.dt.float32, mybir.dt.bfloat16, mybir.dt.int16
    f8 = mybir.dt.float8e4
    AF = mybir.ActivationFunctionType
    ALU = mybir.AluOpType

    wpc, nch, cwins = cfg.wpc, cfg.nch, cfg.cwins
    cwmax = cfg.cwmax
    npc, npad, g = cfg.npc, cfg.npad, cfg.g
    cum = np.concatenate([[0], np.cumsum(cwins)]).astype(int)
    lgdt = bf16 if LG_BF16 else f32

    nc = bacc.Bacc("TRN2", num_swdge_queues=4)

    # ---- I/O
    xT = nc.dram_tensor("xT", [IN, npc], bf16, kind="ExternalInput")
    xTf = nc.dram_tensor("xTf", [IN, NTAB], bf16, kind="ExternalInput")
    MDT = nc.dram_tensor("MDT", [128, nch, 2, 128], f8, kind="ExternalInput")
    srcidx1 = nc.dram_tensor("srcidx1", [128, nch * 8], i16, kind="ExternalInput")
    srcidx2 = nc.dram_tensor("srcidx2", [128, nch * 8], i16, kind="ExternalInput")
    b1hot = nc.dram_tensor("b1hot", [wpc, 128, g], bf16, kind="ExternalInput")
    w1l = nc.dram_tensor("w1l", [IN, HID], bf16, kind="ExternalInput")
    w1r = nc.dram_tensor("w1r", [IN, HID], bf16, kind="ExternalInput")
    w1e_t = nc.dram_tensor("w1e_t", [EDGE_D, wpc, HID], bf16, kind="ExternalInput")
    att1bc = nc.dram_tensor("att1bc", [128, 4, HID], bf16, kind="ExternalInput")
    w2l = nc.dram_tensor("w2l", [HID, HID], bf16, kind="ExternalInput")
    w2r = nc.dram_tensor("w2r", [HID, HID], bf16, kind="ExternalInput")
    w2e_t = nc.dram_tensor("w2e_t", [EDGE_D, wpc, HID], bf16, kind="ExternalInput")
    att2bc = nc.dram_tensor("att2bc", [128, 4, HID], bf16, kind="ExternalInput")
    i128bf = nc.dram_tensor("i128bf", [128, 128], bf16, kind="ExternalInput")
    i128f = nc.dram_tensor("i128f", [128, 128], f32, kind="ExternalInput")
    wpv = nc.dram_tensor("wpv", [HID, ACT + 1], f32, kind="ExternalInput")
    bpv = nc.dram_tensor("bpv", [g, ACT + 1], f32, kind="ExternalInput")
    out = nc.dram_tensor("out", [g, ACT + 1], f32, kind="ExternalOutput")

    # ---- internal DRAM (gather tables in fp8: halves gather traffic)
    xl1_full = nc.dram_tensor("xl1_full", [NTAB, HID], f8)
    xl2_slice = nc.dram_tensor("xl2_slice", [npc, HID], f8)
    xl2_full = nc.dram_tensor("xl2_full", [npad, HID], f8, addr_space="Shared")
    pp_in = nc.dram_tensor("pp_in", [g, HID + 1], f32)
    pp_out = nc.dram_tensor("pp_out", [g, HID + 1], f32, addr_space="Shared")

    taps = {}
    if cfg.debug_taps:
        taps["t_xl1full"] = nc.dram_tensor(
            "t_xl1full", [NTAB, HID], f8, kind="ExternalOutput"
        )
        taps["t_h1"] = nc.dram_tensor(
            "t_h1", [wpc, 128, HID], bf16, kind="ExternalOutput"
        )
        taps["t_pp"] = nc.dram_tensor(
            "t_pp", [g, HID + 1], f32, kind="ExternalOutput"
        )

    rg = [list(range(cfg.ncores))]

    with tile.TileContext(nc) as tc, ExitStack() as ctx:
        nc.gpsimd.load_library(library_config.mlp)

        consts = ctx.enter_context(tc.tile_pool(name="consts", bufs=1))
        # resident constants
        sb_att1 = consts.tile([128, 4, HID], bf16)
        nc.sync.dma_start(out=sb_att1, in_=att1bc[:, :, :])
        sb_att2 = consts.tile([128, 4, HID], bf16)
        nc.sync.dma_start(out=sb_att2, in_=att2bc[:, :, :])
        sb_ibf = consts.tile([128, 128], bf16)
        nc.sync.dma_start(out=sb_ibf, in_=i128bf[:, :])
        sb_if = consts.tile([128, 128], f32)
        nc.sync.dma_start(out=sb_if, in_=i128f[:, :])
        sb_w1l = consts.tile([IN, HID], bf16)
        nc.sync.dma_start(out=sb_w1l, in_=w1l[:, :])
        sb_w1r = consts.tile([IN, HID], bf16)
        nc.sync.dma_start(out=sb_w1r, in_=w1r[:, :])
        sb_w2l = consts.tile([128, 2, HID], bf16)
        nc.sync.dma_start(out=sb_w2l, in_=w2l.rearrange("(j p) f -> p j f", p=128))
        sb_w2r = consts.tile([128, 2, HID], bf16)
        nc.sync.dma_start(out=sb_w2r, in_=w2r.rearrange("(j p) f -> p j f", p=128))
        sb_idx1 = consts.tile([128, nch * 8], i16)
        nc.sync.dma_start(out=sb_idx1, in_=srcidx1[:, :])
        sb_idx2 = consts.tile([128, nch * 8], i16)
        nc.sync.dma_start(out=sb_idx2, in_=srcidx2[:, :])
        sb_b1h = consts.tile([128, wpc, g], bf16)
        nc.sync.dma_start(out=sb_b1h, in_=b1hot.rearrange("w p g -> p w g"))
        sb_wpv = consts.tile([128, 2, ACT + 1], f32)
        nc.sync.dma_start(out=sb_wpv, in_=wpv.rearrange("(j p) f -> p j f", p=128))
        sb_bpv = consts.tile([g, ACT + 1], f32)
        nc.sync.dma_start(out=sb_bpv, in_=bpv[:, :])
        # resident xr tables (rows 0:NW per window = xr, rows NW:128 = We)
        sb_xr1 = consts.tile([128, wpc, HID], bf16)
        nc.sync.dma_start(out=sb_xr1[NW:128, :, :], in_=w1e_t[:, :, :])
        sb_xr2 = consts.tile([128, wpc, HID], bf16)
        nc.sync.dma_start(out=sb_xr2[NW:128, :, :], in_=w2e_t[:, :, :])
        # xl2 staging tile
        sb_xl2st = consts.tile([128, wpc, HID], f8)

        # ---------------- P1: build xl1 gather table (full, local) + own xr1
        _sid, _ = nc.enter_named_scope("p1", False)
        with tc.tile_pool(name="p1", bufs=1) as p1, tc.tile_pool(
            name="p1st", bufs=4
        ) as p1st, tc.tile_pool(name="p1ps", bufs=4, space="PSUM") as p1ps:
            xt_all = p1.tile([IN, npc], bf16)
            nc.sync.dma_start(out=xt_all, in_=xT[:, :])
            xtf = p1.tile([IN, NTAB], bf16)
            nc.sync.dma_start(out=xtf, in_=xTf[:, :])
            for w in range(wpc):
                ps_r = p1ps.tile([128, HID], f32)
                nc.tensor.matmul(
                    ps_r[0:NW, :],
                    xt_all[:, w * NW : (w + 1) * NW],
                    sb_w1r,
                    start=True,
                    stop=True,
                )
                nc.vector.tensor_copy(sb_xr1[0:NW, w, :], ps_r[0:NW, :])
            # partition-major table: row of node n = (n%128)*NT + n//128
            tab_pm = xl1_full.rearrange("(p t) f -> p t f", p=128)
            for t0 in range(0, NT, TBATCH):
                tb = min(TBATCH, NT - t0)
                st = p1st.tile([128, TBATCH, HID], f8, tag="xlst")
                for ti in range(tb):
                    t = t0 + ti
                    ps_l = p1ps.tile([128, HID], f32)
                    nc.tensor.matmul(
                        ps_l, xtf[:, t * 128 : (t + 1) * 128], sb_w1l,
                        start=True, stop=True,
                    )
                    if t % 2 == 0:
                        nc.scalar.copy(st[:, ti, :], ps_l)
                    else:
                        nc.vector.tensor_copy(st[:, ti, :], ps_l)
                nc.sync.dma_start(
                    out=tab_pm[:, t0 : t0 + tb, :], in_=st[:, 0:tb, :]
                )
        nc.leave_named_scope("p1", _sid, False)
        if cfg.debug_taps:
            nc.sync.dma_start(out=taps["t_xl1full"][:, :], in_=xl1_full[:, :])

        _stops = ["p1", "l1", "l2", "full"]

        def _on(phase):
            return _stops.index(cfg.stop_after) >= _stops.index(phase)

        # ---------------- edge phase (shared for both layers)
        def edge_phase(layer):
            table = xl1_full if layer == 1 else xl2_full
            sb_idx = sb_idx1 if layer == 1 else sb_idx2
            xr_res = sb_xr1 if layer == 1 else sb_xr2
            attbc = sb_att1 if layer == 1 else sb_att2
            nheads = HEADS if layer == 1 else 1
            vw = HID + nheads  # V width / nd width
            lctx = ExitStack()
            ep = lctx.enter_context(tc.tile_pool(name=f"ep{layer}", bufs=5))
            sm = lctx.enter_context(tc.tile_pool(name=f"sm{layer}", bufs=5))
            ps_s = lctx.enter_context(
                tc.tile_pool(name=f"pss{layer}", bufs=3, space="PSUM")
            )
            ps_nd = lctx.enter_context(
                tc.tile_pool(name=f"psnd{layer}", bufs=2, space="PSUM")
            )
            if layer == 1:
                ps_pr = lctx.enter_context(
                    tc.tile_pool(name="pspr", bufs=1, space="PSUM")
                )
                ps_tp = lctx.enter_context(
                    tc.tile_pool(name="pstp", bufs=1, space="PSUM")
                )
                pool_ps = None
            else:
                ps_pool = lctx.enter_context(
                    tc.tile_pool(name="pspool", bufs=1, space="PSUM")
                )
                pool_ps = ps_pool.tile([g, HID + 1], f32)

            # pass A: gather + projections + logits + weighted values (vg)
            def pass_a(w):
                cw = cwins[w]
                ck0 = int(cum[w])
                xlg = ep.tile([128, cwmax, HID], f8, tag="xlg")
                nsplit = min(4, cw)
                gsz = [
                    cw // nsplit + (1 if i < cw % nsplit else 0)
                    for i in range(nsplit)
                ]
                go = 0
                for gi, gs in enumerate(gsz):
                    nc.gpsimd.dma_gather(
                        xlg[:, go : go + gs, :],
                        table[:, :],
                        sb_idx[:, (ck0 + go) * 8 : (ck0 + go + gs) * 8],
                        gs * 128,
                        gs * 128,
                        HID,
                        single_packet=False,
                        queue_num=gi,
                    )
                    go += gs
                mdt = ep.tile([128, cwmax, 2, 128], f8, tag="mdt")
                nc.sync.dma_start(
                    out=mdt[:, 0:cw, :, :], in_=MDT[:, ck0 : ck0 + cw, :, :]
                )
                msb = mdt[:, :, 0, :]
                dsb = mdt[:, :, 1, :]
                rw = xr_res[:, w, :]
                vg = sm.tile([128, cwmax, vw], bf16, tag="vg")

                for g0 in range(0, cw, 4):
                    gl = min(4, cw - g0)
                    m_g = sm.tile([128, 4, HID], bf16, tag="m")
                    for p0 in range(0, gl, 2):
                        pl = min(2, gl - p0)
                        ps = ps_s.tile([128, 2, HID], f32)
                        for ci in range(p0, p0 + pl):
                            c = g0 + ci
                            nc.tensor.matmul(
                                ps[:, ci - p0, :],
                                msb[:, c, :],
                                rw,
                                start=True,
                                stop=False,
                            )
                            nc.tensor.matmul(
                                ps[:, ci - p0, :],
                                sb_ibf,
                                xlg[:, c, :],
                                start=False,
                                stop=True,
                            )
                        nc.scalar.activation(
                            m_g[:, p0 : p0 + pl, :],
                            ps[:, 0:pl, :],
                            AF.Prelu,
                            alpha=NEG_SLOPE,
                        )
                    # batched attention logits over the chunk group
                    lgf = sm.tile([128, 4, nheads], lgdt if nheads == 4 else f32,
                                  tag="lg")
                    if nheads == 4:
                        wv_g = sm.tile([128, 4, HID], bf16, tag="wv")
                        nc.vector.tensor_mul(
                            wv_g[:, 0:gl, :], m_g[:, 0:gl, :], attbc[:, 0:gl, :]
                        )
                        with nc.allow_low_precision("logit reduce in bf16 ok"):
                            nc.vector.tensor_reduce(
                                lgf[:, 0:gl, :],
                                wv_g[:, 0:gl, :].rearrange(
                                    "p c (h k) -> p c h k", h=4
                                ),
                                axis=mybir.AxisListType.X,
                                op=ALU.add,
                            )
                    else:
                        wv_g = sm.tile([128, 4, HID], bf16, tag="wv")
                        nc.vector.tensor_mul(
                            wv_g[:, 0:gl, :], m_g[:, 0:gl, :], attbc[:, 0:gl, :]
                        )
                        nc.vector.tensor_reduce(
                            lgf[:, 0:gl, 0],
                            wv_g[:, 0:gl, :],
                            axis=mybir.AxisListType.X,
                            op=ALU.add,
                        )
                    expb = sm.tile([128, 4, nheads], bf16 if nheads == 4 else f32,
                                   tag="expb")
                    nc.scalar.activation(expb[:, 0:gl, :], lgf[:, 0:gl, :], AF.Exp)
                    if nheads == 4:
                        # v = xl[src] * p_h, all heads+chunks in one stride-0
                        # broadcast multiply
                        in0 = xlg[:, g0 : g0 + gl, :].rearrange(
                            "p c (h k) -> p c h k", h=4
                        )
                        in1 = expb[:, 0:gl, :].rearrange(
                            "p c (h k) -> p c h k", h=4
                        )
                        b0, b1 = broadcast_tensor_aps(in0, in1)
                        nc.vector.tensor_mul(
                            vg[:, g0 : g0 + gl, 0:HID].rearrange(
                                "p c (h k) -> p c h k", h=4
                            ),
                            b0,
                            b1,
                        )
                    else:
                        for ci in range(gl):
                            nc.vector.tensor_scalar_mul(
                                vg[:, g0 + ci, 0:HID],
                                xlg[:, g0 + ci, :],
                                expb[:, ci, 0:1],
                            )
                    nc.vector.tensor_copy(
                        vg[:, g0 : g0 + gl, HID:vw], expb[:, 0:gl, :]
                    )
                return dict(cw=cw, dsb=dsb, vg=vg)

            # pass B: scatter-accumulate nd + window epilogue
            def pass_b(w, st):
                cw, dsb, vg = st["cw"], st["dsb"], st["vg"]
                nd = ps_nd.tile([128, vw], f32)
                for c in range(cw):
                    nc.tensor.matmul(
                        nd,
                        dsb[:, c, :],
                        vg[:, c, :],
                        start=(c == 0),
                        stop=(c == cw - 1),
                    )
                dent = sm.tile([128, nheads], f32, tag="dent")
                nc.vector.tensor_scalar_add(dent, nd[:, HID:vw], 1e-16)
                rec = sm.tile([128, nheads], f32, tag="rec")
                nc.vector.reciprocal(rec, dent)
                if layer == 1:
                    hb = sm.tile([128, HID], bf16, tag="hb")
                else:
                    h2v = sm.tile([128, HID + 1], bf16, tag="h2v")
                    hb = h2v[:, 0:HID]
                # h = relu(num * 1/den) on the scalar engine (per-partition
                # scale AP) -- keeps the saturated DVE out of the epilogue
                for h in range(nheads):
                    sl = (
                        slice(h * 64, (h + 1) * 64)
                        if nheads == 4
                        else slice(0, HID)
                    )
                    nc.scalar.activation(
                        hb[:, sl],
                        nd[:, sl],
                        AF.Relu,
                        scale=rec[:, h : h + 1],
                    )
                if layer == 1:
                    if cfg.debug_taps:
                        nc.sync.dma_start(out=taps["t_h1"][w, :, :], in_=hb)
                    h1t = sm.tile([128, 2, 128], bf16, tag="h1t")
                    for j in range(2):
                        tp = ps_tp.tile([128, 128], bf16)
                        nc.tensor.transpose(
                            tp, hb[:, j * 128 : (j + 1) * 128], sb_ibf
                        )
                        nc.vector.tensor_copy(h1t[:, j, :], tp)
                    ps_xl2 = ps_pr.tile([128, HID], f32)
                    ps_xr2 = ps_pr.tile([128, HID], f32)
                    for j in range(2):
                        nc.tensor.matmul(
                            ps_xl2,
                            h1t[:, j, :],
                            sb_w2l[:, j, :],
                            start=(j == 0),
                            stop=(j == 1),
                        )
                        nc.tensor.matmul(
                            ps_xr2,
                            h1t[:, j, :],
                            sb_w2r[:, j, :],
                            start=(j == 0),
                            stop=(j == 1),
                        )
                    nc.scalar.copy(sb_xl2st[0:NW, w, :], ps_xl2[0:NW, :])
                    nc.vector.tensor_copy(sb_xr2[0:NW, w, :], ps_xr2[0:NW, :])
                else:
                    nc.vector.memset(h2v[:, HID : HID + 1], 1.0)
                    if os.environ.get("KERNEL_NO_POOL", "0") != "1":
                        nc.tensor.matmul(
                            pool_ps,
                            sb_b1h[:, w, :],
                            h2v,
                            start=(w == 0),
                            stop=(w == wpc - 1),
                        )

            # after the last pass_b of an ag2 piece (layer 1 only): store the
            # piece's xl2 rows and launch its partial AllGather
            def ag2_piece(p):
                w0, w1 = PB[p], PB[p + 1]
                nc.sync.dma_start(
                    out=xl2_slice.rearrange("(w q) f -> q w f", q=NW)[:, w0:w1, :],
                    in_=sb_xl2st[0:NW, w0:w1, :],
                )
                o0 = cfg.ncores * NW * w0
                o1 = cfg.ncores * NW * w1
                nc.gpsimd.collective_compute(
                    "AllGather",
                    mybir.AluOpType.bypass,
                    ins=[xl2_slice[w0 * NW : w1 * NW, :]],
                    outs=[xl2_full[o0:o1, :]],
                    replica_groups=rg,
                )

            # software pipeline: B(w-1) is emitted after A(w) so the PE can
            # run window w's projection matmuls while window w-1's vg chain
            # finishes on the DVE
            prev = None
            for w in range(wpc):
                st = pass_a(w)
                if prev is not None:
                    pass_b(w - 1, prev)
                    if layer == 1 and w in PB[1:-1]:
                        ag2_piece(PB.index(w) - 1)
                prev = st
            pass_b(wpc - 1, prev)
            if layer == 1:
                ag2_piece(PIECES - 1)

            if layer == 2:
                pps = sm.tile([g, HID + 1], f32, tag="pps")
                nc.scalar.copy(pps, pool_ps)
                nc.sync.dma_start(out=pp_in[:, :], in_=pps)
            lctx.close()

        if _on("l1"):
            _sid, _ = nc.enter_named_scope("l1", False)
            edge_phase(1)
            nc.leave_named_scope("l1", _sid, False)

        if _on("l2"):
            _sid, _ = nc.enter_named_scope("l2", False)
            edge_phase(2)
            nc.leave_named_scope("l2", _sid, False)

        if not _on("full"):
            with tc.tile_pool(name="dummy", bufs=1) as dp:
                ob = dp.tile([g, ACT + 1], f32)
                nc.vector.memset(ob, 0.0)
                nc.sync.dma_start(out=out[:, :], in_=ob)
            nc_done = True
        else:
            nc_done = False

        # ---------------- P6: exchange pool partials (AllToAll with the
        # partial replicated into every block = direct-send AllGather; the
        # ring AllReduce costs ~80us for 65KB)
        if not nc_done:
            _sid, _ = nc.enter_named_scope("ar", False)
            nc.gpsimd.collective_compute(
                "AllReduce",
                mybir.AluOpType.add,
                ins=[pp_in[:, :]],
                outs=[pp_out[:, :]],
                replica_groups=rg,
            )
            nc.leave_named_scope("ar", _sid, False)

        # ---------------- P7: head (replicated)
        if not nc_done:
          with tc.tile_pool(name="hd", bufs=1) as hd, tc.tile_pool(
            name="hdps", bufs=2, space="PSUM"
          ) as hdps:
            pp = hd.tile([g, HID + 1], f32)
            nc.sync.dma_start(out=pp, in_=pp_out[:, :])
            cnt = hd.tile([g, 1], f32)
            nc.vector.tensor_scalar_max(cnt, pp[:, HID : HID + 1], 1.0)
            rcnt = hd.tile([g, 1], f32)
            nc.vector.reciprocal(rcnt, cnt)
            gt = hd.tile([g, HID], f32)
            nc.vector.tensor_scalar_mul(gt, pp[:, 0:HID], rcnt[:, 0:1])
            gT = hd.tile([128, 2, g], f32)
            for j in range(2):
                tp = hdps.tile([128, g], f32)
                nc.tensor.transpose(
                    tp, gt[:, j * 128 : (j + 1) * 128], sb_if[0:g, 0:g]
                )
                nc.vector.tensor_copy(gT[:, j, :], tp)
            ps_o = hdps.tile([g, ACT + 1], f32)
            for j in range(2):
                nc.tensor.matmul(
                    ps_o, gT[:, j, :], sb_wpv[:, j, :], start=(j == 0), stop=(j == 1)
                )
            ob = hd.tile([g, ACT + 1], f32)
            nc.vector.tensor_add(ob, ps_o, sb_bpv)
            nc.sync.dma_start(out=out[:, :], in_=ob)

    nc.finalize()
    return nc


# ================================================================ entry point
_CACHE = {}


def kernel(
    x,
    edge_index,
    batch,
    edge_attr,
    W1l,
    W1r,
    W1e,
    att1,
    b1,
    W2l,
    W2r,
    W2e,
    att2,
    b2,
    Wp,
    bp,
    Wv,
    bv,
    _trace=False,
):
    from concourse.bass_utils import run_bass_kernel_spmd

    cfg = Cfg(stop_after=os.environ.get("STOP_AFTER", "full"))
    in_maps = preprocess(cfg, x, edge_index, batch, edge_attr)
    consts = make_consts(W1l, W1r, W1e, att1, W2l, W2r, W2e, att2, Wp, bp, Wv, bv)
    for m in in_maps:
        m.update(consts)

    key = (tuple(cfg.cwins), cfg.stop_after)
    if key not in _CACHE:
        _CACHE[key] = build_program(cfg)
    nc = _CACHE[key]

    tc_env = os.environ.get("TRACE_CORES", "")
    extra = {}
    if _trace and tc_env:
        extra["trace_cores"] = [int(c) for c in tc_env.split(",")]
    res = run_bass_kernel_spmd(
        nc, in_maps, list(range(cfg.ncores)), trace=_trace, **extra
    )
    out = np.asarray(res.results[0]["out"], dtype=np.float32)
    if _trace:
        kernel.last_exec_time_ns = res.exec_time_ns
        kernel.last_results = res
    return out
